# revision 2
# baseline (speedup 1.0000x reference)
"""Ruleformer MultiHeadAttention TRN2 kernel (relation-biased attention).

Shapes: B=4, N=256, D=256, H=8, DK=DV=32, R=64.
Sharding: 8 cores = (b in 0..4) x (half in 0..2); each core computes
i-rows [half*128, half*128+128) of batch b for all heads.

Math (per core, i in local slice of 128 rows):
  Xq = q @ (Wq/sqrt(DK)), Xk = q @ Wk, Xv = q @ Wv      (per-head views)
  P[h,i,r]   = sum_d XqT[h*32+d, i] * MkT[h*32+d, r]    (MkT = (relationE@Wr).T)
  attn[h,i,j]= sum_d Xq.. + sum_r P[h,i,r]*link[i,j,r] + maskbias
  alpha      = softmax_j(attn)
  S[h,i,r]   = sum_j alpha[h,i,j]*link[i,j,r]
  Z[i,(h,d)] = sum_j alpha[h,i,j]*Xv[j,(h,d)] + sum_r S[h,i,r]*Mv[r,(h,d)]
  out        = LN(Z @ Wo + q_sl) * gamma + beta

Row layout for attention phase: groups g of 128 rows = 8 pairs x (8 h x 2 i),
row (pp, h, ipar) <-> head h, i = g*16 + pp*2 + ipar.
linkT SBUF layout: [ (ipar,r) = 128 part, pair = 64, j = 256 ].
"""

from contextlib import ExitStack

import numpy as np

import concourse.bass as bass
import concourse.mybir as mybir
import concourse.tile as tile
from concourse.masks import make_identity

B, N, D, H, DK, DV, R = 4, 256, 256, 8, 32, 32, 64
EPS = 1e-6
NI = 128          # i-rows per core
NPAIR = NI // 2   # 64 i-pairs
NG = NI // 16     # 8 row-groups of 128 rows (8 pairs * 16)
F32 = mybir.dt.float32
F32R = mybir.dt.float32r
BF16 = mybir.dt.bfloat16

DEFAULT_CFG = dict(
    f32r_big=False,    # use float32r for N>=256 matmuls
    f32r_tr=False,     # use float32r for PE transposes
    link_bf16=False,   # link (and Pblk) in bf16
)


def _mm_dt(ap, on):
    return ap.bitcast(F32R) if on else ap


def build_core_kernel(nc, cfg=None):
    """Declare dram tensors and emit the Tile program. nc: bacc.Bacc."""
    cfg = {**DEFAULT_CFG, **(cfg or {})}
    ldt = BF16 if cfg["link_bf16"] else F32

    def dram(name, shape, dtype=F32, kind="ExternalInput"):
        return nc.dram_tensor(name, shape, dtype, kind=kind).ap()

    q_sl = dram("q_sl", (NI, D))            # residual rows
    qT = dram("qT", (D, N))                 # q[b].T full
    qTi = dram("qTi", (D, NI))              # q[b].T i-slice columns
    maskb = dram("maskb", (NG, 128, N))     # additive mask bias, permuted rows
    linkT = dram("linkT", (NI, R, N), ldt)  # link[b,islice].transpose(0,2,1)
    Wq = dram("Wq", (D, H * DK))            # pre-scaled by DK**-0.5
    Wk = dram("Wk", (D, H * DK))
    Wv = dram("Wv", (D, H * DV))
    MkT = dram("MkT", (H * DK, R))          # (relationE@Wr).T
    Mv = dram("Mv", (R, H * DV))            # relationE@Wvv
    Wo = dram("Wo", (H * DV, D))
    gamma = dram("gamma", (D,))
    beta = dram("beta", (D,))
    alpha_o = dram("alpha_o", (H, NI, N), kind="ExternalOutput")
    out_o = dram("out_o", (NI, D), kind="ExternalOutput")

    with ExitStack() as ctx:
        tc = ctx.enter_context(tile.TileContext(nc))
        sing = ctx.enter_context(tc.tile_pool(name="sing", bufs=1))
        rows = ctx.enter_context(tc.tile_pool(name="rows", bufs=3))
        small = ctx.enter_context(tc.tile_pool(name="small", bufs=4))
        mpool = ctx.enter_context(tc.tile_pool(name="mpool", bufs=2))
        natp = ctx.enter_context(tc.tile_pool(name="natp", bufs=6))
        ps_big = ctx.enter_context(tc.tile_pool(name="ps_big", bufs=2, space="PSUM"))
        ps_sq = ctx.enter_context(tc.tile_pool(name="ps_sq", bufs=4, space="PSUM"))
        ps_s = ctx.enter_context(tc.tile_pool(name="ps_s", bufs=4, space="PSUM"))
        ps_z = ctx.enter_context(tc.tile_pool(name="ps_z", bufs=1, space="PSUM"))
        ps_z2 = ctx.enter_context(tc.tile_pool(name="ps_z2", bufs=1, space="PSUM"))

        ident = sing.tile([128, 128], F32)
        make_identity(nc, ident)

        # --- load weights/constants -------------------------------------
        def load2(ap_dram, w, name):
            ts = [sing.tile([128, w], F32, tag=f"{name}{c}") for c in range(2)]
            for c in range(2):
                nc.sync.dma_start(out=ts[c], in_=ap_dram[c * 128:(c + 1) * 128, :])
            return ts

        qT_sb = load2(qT, N, "qT")
        Wq_sb = load2(Wq, D, "Wq")
        Wk_sb = load2(Wk, D, "Wk")
        Wv_sb = load2(Wv, D, "Wv")
        Wo_sb = load2(Wo, D, "Wo")
        MkT_sb = load2(MkT, R, "MkT")
        qTi_sb = load2(qTi, NI, "qTi")
        Mv_sb = sing.tile([R, H * DV], F32)
        nc.sync.dma_start(out=Mv_sb, in_=Mv)
        qsl_sb = sing.tile([NI, D], F32)
        nc.sync.dma_start(out=qsl_sb, in_=q_sl)
        gamma_bc = sing.tile([128, D], F32)
        nc.gpsimd.dma_start(
            out=gamma_bc,
            in_=bass.AP(tensor=gamma.tensor, offset=0, ap=[[0, 128], [1, D]]),
        )
        beta_bc = sing.tile([128, D], F32)
        nc.gpsimd.dma_start(
            out=beta_bc,
            in_=bass.AP(tensor=beta.tensor, offset=0, ap=[[0, 128], [1, D]]),
        )
        eps_sb = sing.tile([128, 1], F32)
        nc.vector.memset(eps_sb, EPS)

        # --- linkT load: [128=(ipar,r), pair, j] -------------------------
        link_sb = sing.tile([128, NPAIR, N], ldt)
        linkT_r = linkT.rearrange("(pr i2) r j -> (i2 r) pr j", i2=2)
        for g in range(NG):
            nc.sync.dma_start(
                out=link_sb[:, g * 8:(g + 1) * 8, :],
                in_=linkT_r[:, g * 8:(g + 1) * 8, :],
            )

        # --- projections -------------------------------------------------
        # XkT[e,j] = sum_d Wk[d,e]*qT[d,j]
        XkT_sb = [sing.tile([128, N], F32, tag=f"XkT{c}") for c in range(2)]
        for ec in range(2):
            ps = ps_big.tile([128, N], F32, tag="ps_proj")
            for dc in range(2):
                nc.tensor.matmul(
                    ps,
                    _mm_dt(Wk_sb[dc][:, ec * 128:(ec + 1) * 128], cfg["f32r_big"]),
                    _mm_dt(qT_sb[dc], cfg["f32r_big"]),
                    start=(dc == 0), stop=(dc == 1),
                )
            nc.any.tensor_copy(XkT_sb[ec], ps)
        # XqT (i-slice cols only)
        XqT_sb = [sing.tile([128, NI], F32, tag=f"XqT{c}") for c in range(2)]
        for ec in range(2):
            ps = ps_sq.tile([128, NI], F32, tag="ps_xq")
            for dc in range(2):
                nc.tensor.matmul(
                    ps,
                    Wq_sb[dc][:, ec * 128:(ec + 1) * 128],
                    qTi_sb[dc],
                    start=(dc == 0), stop=(dc == 1),
                )
            nc.any.tensor_copy(XqT_sb[ec], ps)
        # Xv natural [j, e]
        Xv_sb = [sing.tile([128, H * DV], F32, tag=f"Xv{c}") for c in range(2)]
        for jc in range(2):
            ps = ps_big.tile([128, H * DV], F32, tag="ps_proj")
            for dc in range(2):
                nc.tensor.matmul(
                    ps,
                    _mm_dt(qT_sb[dc][:, jc * 128:(jc + 1) * 128], cfg["f32r_big"]),
                    _mm_dt(Wv_sb[dc], cfg["f32r_big"]),
                    start=(dc == 0), stop=(dc == 1),
                )
            nc.any.tensor_copy(Xv_sb[jc], ps)

        # --- P: PT_h[r, i] -> PT_sb [64, h, 128] -------------------------
        PT_sb = sing.tile([R, H, NI], F32)
        for h in range(H):
            t, o = h // 4, (h % 4) * 32
            ps = ps_s.tile([R, NI], F32, tag="ps_pt")
            nc.tensor.matmul(
                ps,
                MkT_sb[t][o:o + 32, :],
                XqT_sb[t][o:o + 32, :],
                start=True, stop=True,
                tile_position=(o, 0),
            )
            nc.any.tensor_copy(PT_sb[:, h, :], ps)

        # --- Pblk [128=(ipar,r), pair, slot=(h,ipar)] --------------------
        Pblk = sing.tile([128, NPAIR, 16], ldt)
        nc.vector.memset(Pblk, 0.0)
        for h in range(H):
            for ipar in range(2):
                nc.any.tensor_copy(
                    out=Pblk[ipar * 64:(ipar + 1) * 64, :, h * 2 + ipar],
                    in_=PT_sb[:, h, ipar::2],
                )

        # --- XB blockdiag [128=(h%4,dk), g, pp, slot] --------------------
        XB_sb = []
        for t in range(2):
            xb = sing.tile([128, NG, 8, 16], ldt if False else F32, tag=f"XB{t}")
            nc.vector.memset(xb, 0.0)
            XB_sb.append(xb)
        for h in range(H):
            t, o = h // 4, (h % 4) * 32
            # XqT free is i = (g, pp, ipar); out free dims (g, pp, ipar)
            nc.any.tensor_copy(
                out=XB_sb[t][o:o + 32, :, :, h * 2:h * 2 + 2],
                in_=XqT_sb[t][o:o + 32, :].rearrange("p (g pp i2) -> p g pp i2", g=NG, pp=8),
            )

        # --- attention groups --------------------------------------------
        alphaT_sb = [sing.tile([128, H * NI], F32, tag=f"aT{c}") for c in range(2)]
        alpha_or = alpha_o.rearrange("h (g pp i2) j -> g pp h i2 j", pp=8, i2=2)
        for g in range(NG):
            psa = ps_big.tile([128, N], F32, tag="ps_attn")
            for pp in range(8):
                p = g * 8 + pp
                nc.tensor.matmul(
                    psa[pp * 16:(pp + 1) * 16, :],
                    _mm_dt(Pblk[:, p, :], cfg["f32r_big"] and not cfg["link_bf16"]),
                    _mm_dt(link_sb[:, p, :], cfg["f32r_big"] and not cfg["link_bf16"]),
                    start=True, stop=False,
                    skip_group_check=True,
                )
            for t in range(2):
                nc.tensor.matmul(
                    psa,
                    _mm_dt(
                        XB_sb[t][:, g, :, :], cfg["f32r_big"]),
                    _mm_dt(XkT_sb[t], cfg["f32r_big"]),
                    start=False, stop=(t == 1),
                    skip_group_check=True,
                )
            mb = mpool.tile([128, N], F32, tag="maskb")
            nc.sync.dma_start(out=mb, in_=maskb[g])
            att = rows.tile([128, N], F32, tag="att")
            nc.vector.tensor_add(att, psa, mb)
            nmax = small.tile([128, 1], F32, tag="nmax")
            nc.vector.reduce_max(nmax, att, axis=mybir.AxisListType.X, negate=True)
            e = rows.tile([128, N], F32, tag="e")
            lsum = small.tile([128, 1], F32, tag="lsum")
            nc.scalar.activation(
                e, att, mybir.ActivationFunctionType.Exp,
                bias=nmax, scale=1.0, accum_out=lsum,
            )
            rl = small.tile([128, 1], F32, tag="rl")
            nc.vector.reciprocal(rl, lsum)
            alp = rows.tile([128, N], F32, tag="alp")
            nc.vector.tensor_scalar_mul(alp, e, rl)
            nc.sync.dma_start(out=alpha_or[g], in_=alp)
            for jc in range(2):
                pst = ps_sq.tile([128, 128], F32, tag="ps_aT")
                nc.tensor.transpose(pst, alp[:, jc * 128:(jc + 1) * 128], ident)
                nc.any.tensor_copy(
                    alphaT_sb[jc][:, g * 128:(g + 1) * 128], pst)

        # --- phase 2: S via transposed link pairs ------------------------
        # S_T [64=r, h, i]
        ST_sb = sing.tile([R, H, NI], F32)
        lident = ident
        if cfg["link_bf16"]:
            lident = sing.tile([128, 128], BF16)
            nc.any.tensor_copy(lident, ident)
        for p in range(NPAIR):
            pss = ps_s.tile([128, 16], F32, tag="ps_S")
            for jc in range(2):
                psn = ps_sq.tile([128, 128], ldt, tag="ps_nat")
                nc.tensor.transpose(
                    psn, link_sb[:, p, jc * 128:(jc + 1) * 128], lident)
                nat = natp.tile([128, 128], ldt, tag="nat")
                nc.any.tensor_copy(nat, psn)
                nc.tensor.matmul(
                    pss,
                    _mm_dt(nat, False),
                    _mm_dt(alphaT_sb[jc][:, p * 16:(p + 1) * 16].bitcast(ldt)
                           if cfg["link_bf16"] else
                           alphaT_sb[jc][:, p * 16:(p + 1) * 16], False),
                    start=(jc == 0), stop=(jc == 1),
                )
            for ipar in range(2):
                nc.any.tensor_copy(
                    out=ST_sb[:, :, p * 2 + ipar],
                    in_=pss[ipar * 64:(ipar + 1) * 64, ipar::2],
                )

        # --- ZT = Z1 + Z2, computed transposed: psZT[e, i] ---------------
        aT_v = [alphaT_sb[jc].rearrange(
            "p (g qq h iq) -> p g qq h iq", qq=4, h=H, iq=4) for jc in range(2)]
        psZT = [ps_z.tile([128, NI], F32, name="psZT0"),
                ps_z2.tile([128, NI], F32, name="psZT1")]
        for h in range(H):
            t, o = h // 4, (h % 4) * 32
            tp = (0, 96) if h % 4 == 3 else None
            for jc in range(2):
                nc.tensor.matmul(
                    psZT[t][o:o + 32, :],
                    Xv_sb[jc][:, h * DV:(h + 1) * DV],
                    aT_v[jc][:, :, :, h, :],
                    start=(jc == 0), stop=False,
                    skip_group_check=True, tile_position=tp,
                )
            nc.tensor.matmul(
                psZT[t][o:o + 32, :],
                Mv_sb[:, h * DV:(h + 1) * DV],
                ST_sb[:, h, :],
                start=False, stop=True,
                skip_group_check=True, tile_position=tp,
            )
        ZT_sb = [sing.tile([128, NI], F32, tag=f"ZT{c}", name=f"ZT{c}") for c in range(2)]
        for ec in range(2):
            nc.any.tensor_copy(ZT_sb[ec], psZT[ec])

        # --- out projection, residual, layernorm -------------------------
        pso = ps_big.tile([NI, D], F32, tag="ps_out")
        for ec in range(2):
            nc.tensor.matmul(
                pso,
                _mm_dt(ZT_sb[ec], cfg["f32r_big"]),
                _mm_dt(Wo_sb[ec], cfg["f32r_big"]),
                start=(ec == 0), stop=(ec == 1),
            )
        o_sb = sing.tile([NI, D], F32)
        nc.vector.tensor_add(o_sb, pso, qsl_sb)
        stats = small.tile([NI, 6], F32, tag="stats")
        nc.vector.bn_stats(stats, o_sb)
        mv2 = small.tile([NI, 2], F32, tag="mv2")
        nc.vector.bn_aggr(mv2, stats)
        sd = small.tile([NI, 1], F32, tag="sd")
        nc.scalar.activation(
            sd, mv2[:, 1:2], mybir.ActivationFunctionType.Sqrt,
            bias=eps_sb, scale=1.0)
        rs = small.tile([NI, 1], F32, tag="rs")
        nc.vector.reciprocal(rs, sd)
        xn = sing.tile([NI, D], F32)
        nc.vector.tensor_scalar(
            xn, o_sb, mv2[:, 0:1], rs,
            op0=mybir.AluOpType.subtract, op1=mybir.AluOpType.mult)
        xg = sing.tile([NI, D], F32)
        nc.vector.tensor_mul(xg, xn, gamma_bc)
        xb2 = sing.tile([NI, D], F32)
        nc.vector.tensor_add(xb2, xg, beta_bc)
        nc.sync.dma_start(out=out_o, in_=xb2)


def host_prep(q, k, v, mask, link, Wq, Wk, Wr, Wv, Wvv, relationE, Wo,
              gamma, beta, cfg=None):
    """Full inputs -> list of 8 per-core input dicts."""
    cfg = {**DEFAULT_CFG, **(cfg or {})}
    ldt = np.dtype(np.float32) if not cfg["link_bf16"] else None
    f32 = np.float32
    q = np.asarray(q, f32)
    mask = np.asarray(mask)
    link = np.asarray(link, f32)
    Wq_s = (np.asarray(Wq, f32) * (DK ** -0.5)).copy()
    MkT = np.ascontiguousarray((np.asarray(relationE, f32) @ np.asarray(Wr, f32)).T)
    Mv = np.ascontiguousarray(np.asarray(relationE, f32) @ np.asarray(Wvv, f32))
    in_maps = []
    for c in range(8):
        b, half = c // 2, c % 2
        i0 = half * NI
        q_sl = np.ascontiguousarray(q[b, i0:i0 + NI])
        qT = np.ascontiguousarray(q[b].T)
        qTi = np.ascontiguousarray(q[b].T[:, i0:i0 + NI])
        mb = np.where(mask[b, i0:i0 + NI] == 0, np.float32(-1e9), np.float32(0.0))
        # rows (g, pp, h, ipar) <- i = g*16 + pp*2 + ipar
        mbp = np.broadcast_to(
            mb.reshape(NG, 8, 1, 2, N), (NG, 8, H, 2, N)).reshape(NG, 128, N)
        lT = np.ascontiguousarray(link[b, i0:i0 + NI].transpose(0, 2, 1))
        if cfg["link_bf16"]:
            import ml_dtypes
            lT = lT.astype(ml_dtypes.bfloat16)
        in_maps.append(dict(
            q_sl=q_sl, qT=qT, qTi=qTi,
            maskb=np.ascontiguousarray(mbp), linkT=lT,
            Wq=Wq_s, Wk=np.asarray(Wk, f32), Wv=np.asarray(Wv, f32),
            MkT=MkT, Mv=Mv, Wo=np.asarray(Wo, f32),
            gamma=np.asarray(gamma, f32), beta=np.asarray(beta, f32),
        ))
    return in_maps


def assemble(results):
    """8 per-core result dicts -> (out, alpha) full arrays."""
    out = np.zeros((B, N, D), np.float32)
    alpha = np.zeros((B, H, N, N), np.float32)
    for c, r in enumerate(results):
        b, half = c // 2, c % 2
        i0 = half * NI
        out[b, i0:i0 + NI] = r["out_o"]
        alpha[b, :, i0:i0 + NI, :] = r["alpha_o"]
    return out, alpha


_CACHED = {}


def _get_nc(cfg_key=None):
    if "nc" not in _CACHED:
        import concourse.bacc as bacc
        nc = bacc.Bacc("TRN2", target_bir_lowering=False, debug=False)
        build_core_kernel(nc, _CACHED.get("cfg"))
        nc.compile()
        _CACHED["nc"] = nc
    return _CACHED["nc"]


def kernel(**inputs):
    """Full (unsharded) inputs -> (out, alpha), matching reference()."""
    from concourse.bass_utils import run_bass_kernel_spmd

    in_maps = host_prep(**inputs, cfg=_CACHED.get("cfg"))
    nc = _get_nc()
    res = run_bass_kernel_spmd(nc, in_maps, core_ids=list(range(8)))
    return assemble(res.results)
